# revision 7
# baseline (speedup 1.0000x reference)
"""Trainium2 Bass kernel for nn_Attention_81870666597078.

Multi-head causal self-attention (b=4, s=2048, d=1024, 16 heads) with QKV/O
projections, tensor-parallel over heads: each of the 8 NeuronCores computes
2 heads (128 of the 1024 hidden dims) end-to-end and produces a partial O
projection; the host sums the 8 partials (the "all-reduce").

Per-core dataflow (all matmuls in float32r, out = lhsT.T @ rhs):
  - QKV projection into transposed layout: qT/kT/vT [128 dims, seq] from
    xT tiles (moving) and W^T tiles (stationary).
  - vT is re-transposed on the PE into v-natural [seq, dh] tiles, stored with
    a fused ones-column ([v | 1] per k-tile) so the PV matmul also produces
    the softmax denominator as output row 64.
  - Scores are computed transposed, S^T [k, q], so the PV contraction (over
    k) needs no transposes: softmax = exp on ACT (no max subtraction: scores
    are ~N(0,1) after the 1/8 scale, exp can't overflow), causal mask as a
    multiplicative 0/1 mask on the 4 diagonal k-tiles of each q-tile, and
    fully-masked column ranges of diagonal tiles skipped via slicing.
  - Normalization: reciprocal of the denominator row, broadcast across 64
    partitions via a K=1 ones matmul, multiplied into the PV output.
  - O projection from attn^T tiles (stationary) and W_o^T tiles (moving);
    partial [seq, 1024] output DMAd out.
"""
import os

import numpy as np

import concourse.bass as bass  # noqa: F401
import concourse.mybir as mybir
from concourse import bacc
from concourse.bass_utils import run_bass_kernel_spmd
from concourse.masks import make_identity
from concourse.tile import TileContext

dt = mybir.dt
F32 = dt.float32
F32R = dt.float32r
Exp = mybir.ActivationFunctionType.Exp

N_CORES = 8
B = 4
S = 2048
D = 1024
DH = 64
CD = 128          # head dims per core (2 heads x 64)
NDT = D // 128    # 8 k-tiles over the model dim
NST = S // 512    # 4 seq tiles of 512 per batch
SEQ = B * S       # 8192


def _build_bass():
    nc = bacc.Bacc("TRN2", target_bir_lowering=False, debug=False)
    xt = nc.dram_tensor("xt", [D, SEQ], F32, kind="ExternalInput")
    wqkvt = nc.dram_tensor("wqkvt", [D, 3 * CD], F32, kind="ExternalInput")
    wot = nc.dram_tensor("wot", [CD, D], F32, kind="ExternalInput")
    mask = nc.dram_tensor("mask", [128, 896], F32, kind="ExternalInput")
    onesd = nc.dram_tensor("onesd", [128, 64], F32, kind="ExternalInput")
    out = nc.dram_tensor("out", [SEQ, D], F32, kind="ExternalOutput")

    xt_view = xt.ap().rearrange("(a p) s -> p a s", p=128)      # [128, 8, 8192]
    wq_view = wqkvt.ap().rearrange("(a p) m -> p a m", p=128)   # [128, 8, 384]

    with TileContext(nc) as tc:
        with (
            tc.tile_pool(name="const", bufs=1) as const,
            tc.tile_pool(name="perb", bufs=2) as perb,
            tc.tile_pool(name="xp", bufs=3) as xp,
            tc.tile_pool(name="probs", bufs=4) as probsp,
            tc.tile_pool(name="outp", bufs=4) as outp,
            tc.tile_pool(name="small", bufs=2) as small,
            tc.tile_pool(name="psA", bufs=3, space="PSUM") as psA,
            tc.tile_pool(name="psS", bufs=3, space="PSUM") as psS,
            tc.tile_pool(name="psPV", bufs=2, space="PSUM") as psPV,
        ):
            wq_sb = const.tile([128, NDT, 3 * CD], F32R, tag="wq")
            wot_sb = const.tile([128, D], F32R, tag="wot")
            mask_sb = const.tile([128, 896], F32, tag="mask")
            ident_sb = const.tile([128, 128], F32, tag="ident")
            ones_sb = const.tile([128, 64], F32R, tag="ones")
            nc.sync.dma_start(wq_sb[:], wq_view.bitcast(F32R))
            nc.sync.dma_start(wot_sb[:], wot.ap().bitcast(F32R))
            nc.sync.dma_start(mask_sb[:], mask.ap())
            nc.sync.dma_start(ones_sb[:], onesd.ap().bitcast(F32R))
            make_identity(nc, ident_sb[:])

            for b in range(B):
                qT = perb.tile([128, S], F32R, tag="qT")
                kT = perb.tile([128, S], F32R, tag="kT")
                vT = perb.tile([128, S], F32, tag="vT")
                v65 = perb.tile([128, (S // 128) * 2 * 65], F32R, tag="v65")
                aoT = perb.tile([128, S], F32R, tag="aoT")
                # ones column of every [v | 1] group (32 groups of 65 cols)
                v65g = v65[:].rearrange("p (g c) -> p g c", c=65)
                nc.vector.tensor_copy(
                    v65g[:, :, 64:65],
                    ones_sb[:, 0:1][:, None, :].broadcast_to([128, 32, 1]))

                # ---- QKV projection (into transposed [dims, seq] layout) ----
                for st in range(NST):
                    xtile = xp.tile([128, NDT, 512], F32R, tag="xt")
                    c = b * S + st * 512
                    nc.sync.dma_start(xtile[:], xt_view[:, :, c:c + 512].bitcast(F32R))
                    for g, dest in ((0, qT), (1, kT), (2, vT)):
                        psp = psA.tile([128, 512], F32, tag="psA")
                        for kt in range(NDT):
                            nc.tensor.matmul(
                                psp[:],
                                wq_sb[:, kt, g * 128:(g + 1) * 128],
                                xtile[:, kt, :],
                                start=(kt == 0), stop=(kt == NDT - 1),
                            )
                        nc.scalar.copy(dest[:, st * 512:(st + 1) * 512], psp[:])

                # ---- v natural layout [seq, dh] with fused ones column ----
                for t in range(S // 128):
                    pst = psA.tile([128, 128], F32, tag="psA")
                    nc.tensor.transpose(pst[:], vT[:, t * 128:(t + 1) * 128],
                                        ident_sb[:])
                    for h in (0, 1):
                        g0 = (t * 2 + h) * 65
                        nc.vector.tensor_copy(v65[:, g0:g0 + 64],
                                              pst[:, h * 64:(h + 1) * 64])

                # ---- attention + O projection per 512-wide q tile ----
                for qt in range(NST):
                    for h in (0, 1):
                        pv = psPV.tile([65, 512], F32, tag="pv")
                        nkt = 4 * (qt + 1)
                        for kt in range(nkt):
                            o = kt * 128 - qt * 512
                            c0 = max(0, o)
                            sp = psS.tile([128, 512], F32, tag="s")
                            nc.tensor.matmul(
                                sp[:, c0:512],
                                kT[h * 64:(h + 1) * 64, kt * 128:(kt + 1) * 128],
                                qT[h * 64:(h + 1) * 64,
                                   qt * 512 + c0:(qt + 1) * 512],
                                start=True, stop=True,
                            )
                            pr = probsp.tile([128, 512], F32R, tag="pr")
                            nc.scalar.activation(pr[:, c0:512], sp[:, c0:512],
                                                 Exp, scale=0.125)
                            if o >= 0:
                                nc.vector.tensor_mul(pr[:, c0:512], pr[:, c0:512],
                                                     mask_sb[:, 384:896 - o])
                            g0 = (kt * 2 + h) * 65
                            nc.tensor.matmul(
                                pv[:, c0:512],
                                v65[:, g0:g0 + 65],
                                pr[:, c0:512],
                                start=(kt == 0), stop=(kt == nkt - 1),
                                skip_group_check=True,
                            )
                        # normalize by the denominator (PV row 64)
                        rc = small.tile([1, 512], F32R, tag="rc")
                        with nc.allow_low_precision(
                                reason="f32r reciprocal: ~1e-4 rounding is fine"):
                            nc.vector.reciprocal(rc[:], pv[64:65, :])
                        pbc = psA.tile([64, 512], F32, tag="psA")
                        nc.tensor.matmul(pbc[:], ones_sb[0:1, :], rc[:],
                                         start=True, stop=True)
                        rb = small.tile([64, 512], F32, tag="rb")
                        nc.scalar.copy(rb[:], pbc[:])
                        nc.vector.tensor_mul(
                            aoT[h * 64:(h + 1) * 64, qt * 512:(qt + 1) * 512],
                            pv[0:64, :], rb[:])

                    # O projection for this q block
                    for t in range(4):
                        tt = qt * 4 + t
                        for ot in range(2):
                            po = psA.tile([128, 512], F32, tag="psA")
                            nc.tensor.matmul(
                                po[:],
                                aoT[:, tt * 128:(tt + 1) * 128],
                                wot_sb[:, ot * 512:(ot + 1) * 512],
                                start=True, stop=True,
                            )
                            ob = outp.tile([128, 512], F32, tag="ob")
                            nc.vector.tensor_copy(ob[:], po[:])
                            r0 = b * S + tt * 128
                            nc.sync.dma_start(
                                out.ap()[r0:r0 + 128, ot * 512:(ot + 1) * 512],
                                ob[:])
    nc.compile()
    return nc


def _causal_mask():
    r = np.arange(128, dtype=np.int64)[:, None]
    j = np.arange(896, dtype=np.int64)[None, :]
    return (r <= j - 384).astype(np.float32)


def _maybe_register_ntff_hook():
    try:
        import antenv
        if getattr(antenv, "axon_hooks", None) is not None:
            return True
        import sys
        import types
        from trn_agent_boot.trn_boot import _ntff_profile_via_ctypes
        mod = types.ModuleType("antenv.axon_hooks")
        state = {"hook": _ntff_profile_via_ctypes("/opt/axon/libaxon_pjrt.so")}
        mod.set_axon_ntff_profile_hook = lambda h: state.__setitem__("hook", h)
        mod.get_axon_ntff_profile_hook = lambda: state["hook"]
        sys.modules["antenv.axon_hooks"] = mod
        antenv.axon_hooks = mod
        return True
    except Exception:
        return False


_NC_CACHE = {}


def kernel(x, W_qkv, W_o):
    assert x.shape == (B, S, D)
    xt = np.ascontiguousarray(x.reshape(B * S, D).T).astype(np.float32)
    mask = _causal_mask()
    in_maps = []
    for c in range(N_CORES):
        sl = slice(c * CD, (c + 1) * CD)
        wqkv_c = np.concatenate(
            [W_qkv[0 * D:][sl], W_qkv[1 * D:][sl], W_qkv[2 * D:][sl]], axis=0)
        wqkvt = np.ascontiguousarray(wqkv_c.T).astype(np.float32)
        wot = np.ascontiguousarray(W_o[:, sl].T).astype(np.float32)
        in_maps.append({"xt": xt, "wqkvt": wqkvt, "wot": wot, "mask": mask,
                        "onesd": np.ones((128, 64), dtype=np.float32)})

    if "nc" not in _NC_CACHE:
        _NC_CACHE["nc"] = _build_bass()
    nc = _NC_CACHE["nc"]

    trace = bool(os.environ.get("BASS_KERNEL_TRACE")) and _maybe_register_ntff_hook()
    res = run_bass_kernel_spmd(nc, in_maps, core_ids=list(range(N_CORES)),
                               trace=trace)
    if trace and res.exec_time_ns is not None:
        print(f"HW exec time: {res.exec_time_ns} ns")

    acc = np.zeros((SEQ, D), dtype=np.float64)
    for c in range(N_CORES):
        acc += res.results[c]["out"]
    return acc.astype(np.float32).reshape(B, S, D)


# revision 13
# speedup vs baseline: 1.3022x; 1.3022x over previous
"""Trainium2 Bass kernel for nn_Attention_81870666597078.

Multi-head causal self-attention (b=4, s=2048, d=1024, 16 heads) with QKV/O
projections, tensor-parallel over heads: each of the 8 NeuronCores computes
2 heads (128 of the 1024 hidden dims) end-to-end and produces a partial O
projection; the host sums the 8 partials (the "all-reduce").

Per-core dataflow (matmuls in bf16 with fp32 PSUM accumulation; the
softmax-denominator reciprocal path stays fp32):
  - QKV projection into transposed layout: qT/kT/vT [128 dims, seq] from
    xT tiles (moving) and W^T tiles (stationary).
  - vT is re-transposed on the PE into v-natural [seq, dh] tiles, stored with
    a fused ones-column ([v | 1] per k-tile) so the PV matmul also produces
    the softmax denominator as output row 64.
  - Scores are computed transposed, S^T [k, q], so the PV contraction (over
    k) needs no transposes. Score tiles come in pairs sharing a 2-bank
    [128, 1024] PSUM tile so one ACT exp covers both (amortizes the ~352-cycle
    ACTIVATE overhead). No max subtraction: scores are ~N(0,1) after the 1/8
    scale, exp cannot overflow. Causal masking is a multiplicative 0/1
    [128, 128] mask on the diagonal band of each diagonal tile; fully-masked
    column ranges are skipped in the PV matmul via slicing.
  - Normalization: reciprocal_approx_fast of the denominator row, broadcast
    across 64 partitions via a K=1 fp32 ones matmul, multiplied into the PV
    output.
  - O projection from attn^T tiles (stationary) and W_o^T tiles (moving);
    partial [seq, 1024] fp32 output DMAd out.
"""
import os

import numpy as np

import concourse.bass as bass  # noqa: F401
import concourse.mybir as mybir
from concourse import bacc
from concourse.bass_utils import run_bass_kernel_spmd
from concourse.masks import make_identity
from concourse.tile import TileContext

dt = mybir.dt
F32 = dt.float32
BF16 = dt.bfloat16
Exp = mybir.ActivationFunctionType.Exp

N_CORES = 8
B = 4
S = 2048
D = 1024
DH = 64
CD = 128          # head dims per core (2 heads x 64)
NDT = D // 128    # 8 k-tiles over the model dim
NST = S // 512    # 4 seq tiles of 512 per batch
SEQ = B * S       # 8192


def _build_bass():
    nc = bacc.Bacc("TRN2", target_bir_lowering=False, debug=False)
    xt = nc.dram_tensor("xt", [D, SEQ], BF16, kind="ExternalInput")
    wqkvt = nc.dram_tensor("wqkvt", [D, 3 * CD], BF16, kind="ExternalInput")
    wot = nc.dram_tensor("wot", [CD, D], BF16, kind="ExternalInput")
    mask = nc.dram_tensor("mask", [128, 128], BF16, kind="ExternalInput")
    onesd = nc.dram_tensor("onesd", [128, 64], F32, kind="ExternalInput")
    out = nc.dram_tensor("out", [SEQ, D], F32, kind="ExternalOutput")
    dbg = os.environ.get("BASS_KERNEL_DEBUG")
    if dbg:
        dbg_q = nc.dram_tensor("dbg_q", [128, S], BF16, kind="ExternalOutput")
        dbg_k = nc.dram_tensor("dbg_k", [128, S], BF16, kind="ExternalOutput")
        dbg_v = nc.dram_tensor("dbg_v", [128, 2080], BF16, kind="ExternalOutput")
        dbg_ao = nc.dram_tensor("dbg_ao", [128, S], BF16, kind="ExternalOutput")
        dbg_den = nc.dram_tensor("dbg_den", [8, 512], F32, kind="ExternalOutput")
        dbg_rc = nc.dram_tensor("dbg_rc", [8, 512], F32, kind="ExternalOutput")
        dbg_pr = nc.dram_tensor("dbg_pr", [128, 1024], BF16, kind="ExternalOutput")

    xt_view = xt.ap().rearrange("(a p) s -> p a s", p=128)      # [128, 8, 8192]
    wq_view = wqkvt.ap().rearrange("(a p) m -> p a m", p=128)   # [128, 8, 384]

    with TileContext(nc) as tc:
        with (
            tc.tile_pool(name="const", bufs=1) as const,
            tc.tile_pool(name="perb", bufs=2) as perb,
            tc.tile_pool(name="xp", bufs=3) as xp,
            tc.tile_pool(name="probs", bufs=4) as probsp,
            tc.tile_pool(name="outp", bufs=4) as outp,
            tc.tile_pool(name="small", bufs=2) as small,
            tc.tile_pool(name="psA", bufs=2, space="PSUM") as psA,
            tc.tile_pool(name="psS", bufs=2, space="PSUM") as psS,
            tc.tile_pool(name="psPV", bufs=2, space="PSUM") as psPV,
        ):
            wq_sb = const.tile([128, NDT, 3 * CD], BF16, tag="wq")
            wot_sb = const.tile([128, D], BF16, tag="wot")
            mask_sb = const.tile([128, 128], BF16, tag="mask")
            ident_sb = const.tile([128, 128], F32, tag="ident")
            ones_sb = const.tile([128, 64], F32, tag="ones")
            nc.sync.dma_start(wq_sb[:], wq_view)
            nc.sync.dma_start(wot_sb[:], wot.ap())
            nc.sync.dma_start(mask_sb[:], mask.ap())
            nc.sync.dma_start(ones_sb[:], onesd.ap())
            make_identity(nc, ident_sb[:])

            for b in range(B):
                qT = perb.tile([128, S], BF16, tag="qT")
                kT = perb.tile([128, S], BF16, tag="kT")
                vT = perb.tile([128, S], F32, tag="vT")
                v65 = perb.tile([128, (S // 128) * 2 * 65], BF16, tag="v65")
                aoT = perb.tile([128, S], BF16, tag="aoT")
                # ones column of every [v | 1] group (32 groups of 65 cols)
                v65g = v65[:].rearrange("p (g c) -> p g c", c=65)
                nc.vector.tensor_copy(
                    v65g[:, :, 64:65],
                    ones_sb[:, 0:1][:, None, :].broadcast_to([128, 32, 1]))

                # ---- QKV projection (into transposed [dims, seq] layout) ----
                for st in range(NST):
                    xtile = xp.tile([128, NDT, 512], BF16, tag="xt")
                    c = b * S + st * 512
                    nc.sync.dma_start(xtile[:], xt_view[:, :, c:c + 512])
                    for g, dest in ((0, qT), (1, kT), (2, vT)):
                        psp = psA.tile([128, 512], F32, tag="psA")
                        for kt in range(NDT):
                            nc.tensor.matmul(
                                psp[:],
                                wq_sb[:, kt, g * 128:(g + 1) * 128],
                                xtile[:, kt, :],
                                start=(kt == 0), stop=(kt == NDT - 1),
                            )
                        nc.vector.tensor_copy(dest[:, st * 512:(st + 1) * 512],
                                              psp[:])

                # ---- v natural layout [seq, dh] with fused ones column ----
                for t in range(S // 128):
                    pst = psA.tile([128, 128], F32, tag="psA")
                    nc.tensor.transpose(pst[:], vT[:, t * 128:(t + 1) * 128],
                                        ident_sb[:])
                    for h in (0, 1):
                        g0 = (t * 2 + h) * 65
                        nc.vector.tensor_copy(v65[:, g0:g0 + 64],
                                              pst[:, h * 64:(h + 1) * 64])

                if dbg and b == 0:
                    nc.sync.dma_start(dbg_q.ap(), qT[:])
                    nc.sync.dma_start(dbg_k.ap(), kT[:])
                    nc.sync.dma_start(dbg_v.ap(), v65[:])

                # ---- attention + O projection per 512-wide q tile ----
                for qt in range(NST):
                    for h in (0, 1):
                        pv = psPV.tile([65, 512], F32, tag="pv")
                        nkt = 4 * (qt + 1)
                        for kp in range(nkt // 2):
                            sp = psS.tile([128, 1024], F32, tag="s")
                            pr = probsp.tile([128, 1024], BF16, tag="pr")
                            for kl in (0, 1):
                                kt = kp * 2 + kl
                                nc.tensor.matmul(
                                    sp[:, kl * 512:(kl + 1) * 512],
                                    kT[h * 64:(h + 1) * 64,
                                       kt * 128:(kt + 1) * 128],
                                    qT[h * 64:(h + 1) * 64,
                                       qt * 512:(qt + 1) * 512],
                                    start=True, stop=True,
                                )
                            # one exp over both k-tiles (2 PSUM banks wide)
                            nc.scalar.activation(pr[:], sp[:], Exp, scale=0.125)
                            for kl in (0, 1):
                                kt = kp * 2 + kl
                                o = kt * 128 - qt * 512
                                c0 = max(0, o)
                                if o >= 0:
                                    nc.vector.tensor_mul(
                                        pr[:, kl * 512 + o:kl * 512 + o + 128],
                                        pr[:, kl * 512 + o:kl * 512 + o + 128],
                                        mask_sb[:])
                                g0 = (kt * 2 + h) * 65
                                nc.tensor.matmul(
                                    pv[:, c0:512],
                                    v65[:, g0:g0 + 65],
                                    pr[:, kl * 512 + c0:(kl + 1) * 512],
                                    start=(kt == 0), stop=(kt == nkt - 1),
                                    skip_group_check=True,
                                )
                            if dbg and b == 0 and h == 0 and qt == 0 and kp == 0:
                                nc.sync.dma_start(dbg_pr.ap(), pr[:])
                        # normalize by the denominator (PV row 64)
                        den = small.tile([1, 512], F32, tag="den")
                        nc.vector.tensor_copy(den[:], pv[64:65, :])
                        rcf = small.tile([1, 512], F32, tag="rcf")
                        nc.vector.reciprocal_approx_fast(rcf[:], den[:])
                        pbc = psA.tile([64, 512], F32, tag="psA")
                        nc.tensor.matmul(pbc[:], ones_sb[0:1, :], rcf[:],
                                         start=True, stop=True)
                        rb = small.tile([64, 512], F32, tag="rb")
                        nc.scalar.copy(rb[:], pbc[:])
                        a0, a1 = h * 64, (h + 1) * 64
                        q0, q1 = qt * 512, (qt + 1) * 512
                        nc.vector.tensor_mul(aoT[a0:a1, q0:q1], pv[0:64, :],
                                             rb[:])
                        if dbg and b == 0:
                            di = h * 4 + qt
                            nc.sync.dma_start(dbg_den.ap()[di:di + 1, :], den[:])
                            nc.sync.dma_start(dbg_rc.ap()[di:di + 1, :], rcf[:])

                    # O projection for this q block
                    for t in range(4):
                        tt = qt * 4 + t
                        for ot in range(2):
                            po = psA.tile([128, 512], F32, tag="psA")
                            nc.tensor.matmul(
                                po[:],
                                aoT[:, tt * 128:(tt + 1) * 128],
                                wot_sb[:, ot * 512:(ot + 1) * 512],
                                start=True, stop=True,
                            )
                            ob = outp.tile([128, 512], F32, tag="ob")
                            nc.vector.tensor_copy(ob[:], po[:])
                            r0 = b * S + tt * 128
                            nc.sync.dma_start(
                                out.ap()[r0:r0 + 128, ot * 512:(ot + 1) * 512],
                                ob[:])
                if dbg and b == 0:
                    nc.sync.dma_start(dbg_ao.ap(), aoT[:])
    nc.compile()
    return nc


def _causal_mask():
    # mask[r, j] = 1 where the key row r is visible to query column j
    r = np.arange(128)[:, None]
    j = np.arange(128)[None, :]
    return (r <= j).astype(np.float32)


def _maybe_register_ntff_hook():
    try:
        import antenv
        if getattr(antenv, "axon_hooks", None) is not None:
            return True
        import sys
        import types
        from trn_agent_boot.trn_boot import _ntff_profile_via_ctypes
        mod = types.ModuleType("antenv.axon_hooks")
        state = {"hook": _ntff_profile_via_ctypes("/opt/axon/libaxon_pjrt.so")}
        mod.set_axon_ntff_profile_hook = lambda h: state.__setitem__("hook", h)
        mod.get_axon_ntff_profile_hook = lambda: state["hook"]
        sys.modules["antenv.axon_hooks"] = mod
        antenv.axon_hooks = mod
        return True
    except Exception:
        return False


_NC_CACHE = {}


def kernel(x, W_qkv, W_o):
    import ml_dtypes
    bf16 = ml_dtypes.bfloat16
    assert x.shape == (B, S, D)
    xt = np.ascontiguousarray(
        x.reshape(B * S, D).T.astype(np.float32)).astype(bf16)
    mask = _causal_mask().astype(bf16)
    onesd = np.ones((128, 64), dtype=np.float32)
    in_maps = []
    for c in range(N_CORES):
        sl = slice(c * CD, (c + 1) * CD)
        wqkv_c = np.concatenate(
            [W_qkv[0 * D:][sl], W_qkv[1 * D:][sl], W_qkv[2 * D:][sl]], axis=0)
        wqkvt = np.ascontiguousarray(wqkv_c.T.astype(np.float32)).astype(bf16)
        wot = np.ascontiguousarray(W_o[:, sl].T.astype(np.float32)).astype(bf16)
        in_maps.append({"xt": xt, "wqkvt": wqkvt, "wot": wot, "mask": mask,
                        "onesd": onesd})

    if "nc" not in _NC_CACHE:
        _NC_CACHE["nc"] = _build_bass()
    nc = _NC_CACHE["nc"]

    trace = bool(os.environ.get("BASS_KERNEL_TRACE")) and _maybe_register_ntff_hook()
    res = run_bass_kernel_spmd(nc, in_maps, core_ids=list(range(N_CORES)),
                               trace=trace)
    if trace and res.exec_time_ns is not None:
        print(f"HW exec time: {res.exec_time_ns} ns")

    acc = np.zeros((SEQ, D), dtype=np.float64)
    for c in range(N_CORES):
        acc += res.results[c]["out"]
    return acc.astype(np.float32).reshape(B, S, D)


# revision 15
# speedup vs baseline: 1.3080x; 1.0045x over previous
"""Trainium2 Bass kernel for nn_Attention_81870666597078.

Multi-head causal self-attention (b=4, s=2048, d=1024, 16 heads) with QKV/O
projections, tensor-parallel over heads: each of the 8 NeuronCores computes
2 heads (128 of the 1024 hidden dims) end-to-end and produces a partial O
projection; the host sums the 8 partials (the "all-reduce").

Per-core dataflow (matmuls in fp16 with fp32 PSUM accumulation; the
softmax-denominator reciprocal path stays fp32):
  - QKV projection into transposed layout: qT/kT/vT [128 dims, seq] from
    xT tiles (moving) and W^T tiles (stationary).
  - vT is re-transposed on the PE into v-natural [seq, dh] tiles, stored with
    a fused ones-column ([v | 1] per k-tile) so the PV matmul also produces
    the softmax denominator as output row 64.
  - Scores are computed transposed, S^T [k, q], so the PV contraction (over
    k) needs no transposes. Score tiles come in pairs sharing a 2-bank
    [128, 1024] PSUM tile so one ACT exp covers both (amortizes the ~352-cycle
    ACTIVATE overhead). No max subtraction: scores are ~N(0,1) after the 1/8
    scale, exp cannot overflow. Causal masking is a multiplicative 0/1
    [128, 128] mask on the diagonal band of each diagonal tile; fully-masked
    column ranges are skipped in the PV matmul via slicing.
  - Normalization: reciprocal_approx_fast of the denominator row, broadcast
    across 64 partitions via a K=1 fp32 ones matmul, multiplied into the PV
    output.
  - O projection from attn^T tiles (stationary) and W_o^T tiles (moving);
    partial [seq, 1024] fp32 output DMAd out.
"""
import os

import numpy as np

import concourse.bass as bass  # noqa: F401
import concourse.mybir as mybir
from concourse import bacc
from concourse.bass_utils import run_bass_kernel_spmd
from concourse.masks import make_identity
from concourse.tile import TileContext

dt = mybir.dt
F32 = dt.float32
F16 = dt.float16
Exp = mybir.ActivationFunctionType.Exp

N_CORES = 8
B = 4
S = 2048
D = 1024
DH = 64
CD = 128          # head dims per core (2 heads x 64)
NDT = D // 128    # 8 k-tiles over the model dim
NST = S // 512    # 4 seq tiles of 512 per batch
SEQ = B * S       # 8192


def _build_bass():
    nc = bacc.Bacc("TRN2", target_bir_lowering=False, debug=False)
    xt = nc.dram_tensor("xt", [D, SEQ], F16, kind="ExternalInput")
    wqkvt = nc.dram_tensor("wqkvt", [D, 3 * CD], F16, kind="ExternalInput")
    wot = nc.dram_tensor("wot", [CD, D], F16, kind="ExternalInput")
    mask = nc.dram_tensor("mask", [128, 128], F16, kind="ExternalInput")
    onesd = nc.dram_tensor("onesd", [128, 64], F32, kind="ExternalInput")
    out = nc.dram_tensor("out", [SEQ, D], F32, kind="ExternalOutput")
    dbg = os.environ.get("BASS_KERNEL_DEBUG")
    if dbg:
        dbg_q = nc.dram_tensor("dbg_q", [128, S], F16, kind="ExternalOutput")
        dbg_k = nc.dram_tensor("dbg_k", [128, S], F16, kind="ExternalOutput")
        dbg_v = nc.dram_tensor("dbg_v", [128, 2080], F16, kind="ExternalOutput")
        dbg_ao = nc.dram_tensor("dbg_ao", [128, S], F16, kind="ExternalOutput")
        dbg_den = nc.dram_tensor("dbg_den", [8, 512], F32, kind="ExternalOutput")
        dbg_rc = nc.dram_tensor("dbg_rc", [8, 512], F32, kind="ExternalOutput")
        dbg_pr = nc.dram_tensor("dbg_pr", [128, 1024], F16, kind="ExternalOutput")

    xt_view = xt.ap().rearrange("(a p) s -> p a s", p=128)      # [128, 8, 8192]
    wq_view = wqkvt.ap().rearrange("(a p) m -> p a m", p=128)   # [128, 8, 384]

    with TileContext(nc) as tc:
        with (
            tc.tile_pool(name="const", bufs=1) as const,
            tc.tile_pool(name="perb", bufs=2) as perb,
            tc.tile_pool(name="xp", bufs=3) as xp,
            tc.tile_pool(name="probs", bufs=4) as probsp,
            tc.tile_pool(name="outp", bufs=4) as outp,
            tc.tile_pool(name="small", bufs=2) as small,
            tc.tile_pool(name="psA", bufs=2, space="PSUM") as psA,
            tc.tile_pool(name="psS", bufs=2, space="PSUM") as psS,
            tc.tile_pool(name="psPV", bufs=2, space="PSUM") as psPV,
        ):
            wq_sb = const.tile([128, NDT, 3 * CD], F16, tag="wq")
            wot_sb = const.tile([128, D], F16, tag="wot")
            mask_sb = const.tile([128, 128], F16, tag="mask")
            ident_sb = const.tile([128, 128], F32, tag="ident")
            ones_sb = const.tile([128, 64], F32, tag="ones")
            nc.sync.dma_start(wq_sb[:], wq_view)
            nc.sync.dma_start(wot_sb[:], wot.ap())
            nc.sync.dma_start(mask_sb[:], mask.ap())
            nc.sync.dma_start(ones_sb[:], onesd.ap())
            make_identity(nc, ident_sb[:])

            for b in range(B):
                qT = perb.tile([128, S], F16, tag="qT")
                kT = perb.tile([128, S], F16, tag="kT")
                vT = perb.tile([128, S], F32, tag="vT")
                v65 = perb.tile([128, (S // 128) * 2 * 65], F16, tag="v65")
                aoT = perb.tile([128, S], F16, tag="aoT")
                # ones column of every [v | 1] group (32 groups of 65 cols)
                v65g = v65[:].rearrange("p (g c) -> p g c", c=65)
                nc.vector.tensor_copy(
                    v65g[:, :, 64:65],
                    ones_sb[:, 0:1][:, None, :].broadcast_to([128, 32, 1]))

                # ---- QKV projection (into transposed [dims, seq] layout) ----
                for st in range(NST):
                    xtile = xp.tile([128, NDT, 512], F16, tag="xt")
                    c = b * S + st * 512
                    nc.sync.dma_start(xtile[:], xt_view[:, :, c:c + 512])
                    for g, dest in ((0, qT), (1, kT), (2, vT)):
                        psp = psA.tile([128, 512], F32, tag="psA")
                        for kt in range(NDT):
                            nc.tensor.matmul(
                                psp[:],
                                wq_sb[:, kt, g * 128:(g + 1) * 128],
                                xtile[:, kt, :],
                                start=(kt == 0), stop=(kt == NDT - 1),
                            )
                        nc.vector.tensor_copy(dest[:, st * 512:(st + 1) * 512],
                                              psp[:])

                # ---- v natural layout [seq, dh] with fused ones column ----
                for t in range(S // 128):
                    pst = psA.tile([128, 128], F32, tag="psA")
                    nc.tensor.transpose(pst[:], vT[:, t * 128:(t + 1) * 128],
                                        ident_sb[:])
                    for h in (0, 1):
                        g0 = (t * 2 + h) * 65
                        nc.vector.tensor_copy(v65[:, g0:g0 + 64],
                                              pst[:, h * 64:(h + 1) * 64])

                if dbg and b == 0:
                    nc.sync.dma_start(dbg_q.ap(), qT[:])
                    nc.sync.dma_start(dbg_k.ap(), kT[:])
                    nc.sync.dma_start(dbg_v.ap(), v65[:])

                # ---- attention + O projection per 512-wide q tile ----
                for qt in range(NST):
                    for h in (0, 1):
                        pv = psPV.tile([65, 512], F32, tag="pv")
                        nkt = 4 * (qt + 1)
                        for kp in range(nkt // 2):
                            sp = psS.tile([128, 1024], F32, tag="s")
                            pr = probsp.tile([128, 1024], F16, tag="pr")
                            for kl in (0, 1):
                                kt = kp * 2 + kl
                                nc.tensor.matmul(
                                    sp[:, kl * 512:(kl + 1) * 512],
                                    kT[h * 64:(h + 1) * 64,
                                       kt * 128:(kt + 1) * 128],
                                    qT[h * 64:(h + 1) * 64,
                                       qt * 512:(qt + 1) * 512],
                                    start=True, stop=True,
                                )
                            # one exp over both k-tiles (2 PSUM banks wide)
                            nc.scalar.activation(pr[:], sp[:], Exp, scale=0.125)
                            for kl in (0, 1):
                                kt = kp * 2 + kl
                                o = kt * 128 - qt * 512
                                c0 = max(0, o)
                                if o >= 0:
                                    nc.vector.tensor_mul(
                                        pr[:, kl * 512 + o:kl * 512 + o + 128],
                                        pr[:, kl * 512 + o:kl * 512 + o + 128],
                                        mask_sb[:])
                                g0 = (kt * 2 + h) * 65
                                nc.tensor.matmul(
                                    pv[:, c0:512],
                                    v65[:, g0:g0 + 65],
                                    pr[:, kl * 512 + c0:(kl + 1) * 512],
                                    start=(kt == 0), stop=(kt == nkt - 1),
                                    skip_group_check=True,
                                )
                            if dbg and b == 0 and h == 0 and qt == 0 and kp == 0:
                                nc.sync.dma_start(dbg_pr.ap(), pr[:])
                        # normalize by the denominator (PV row 64)
                        den = small.tile([1, 512], F32, tag="den")
                        nc.vector.tensor_copy(den[:], pv[64:65, :])
                        rcf = small.tile([1, 512], F32, tag="rcf")
                        nc.vector.reciprocal_approx_fast(rcf[:], den[:])
                        pbc = psA.tile([64, 512], F32, tag="psA")
                        nc.tensor.matmul(pbc[:], ones_sb[0:1, :], rcf[:],
                                         start=True, stop=True)
                        rb = small.tile([64, 512], F32, tag="rb")
                        nc.scalar.copy(rb[:], pbc[:])
                        a0, a1 = h * 64, (h + 1) * 64
                        q0, q1 = qt * 512, (qt + 1) * 512
                        nc.vector.tensor_mul(aoT[a0:a1, q0:q1], pv[0:64, :],
                                             rb[:])
                        if dbg and b == 0:
                            di = h * 4 + qt
                            nc.sync.dma_start(dbg_den.ap()[di:di + 1, :], den[:])
                            nc.sync.dma_start(dbg_rc.ap()[di:di + 1, :], rcf[:])

                    # O projection for this q block
                    for t in range(4):
                        tt = qt * 4 + t
                        for ot in range(2):
                            po = psA.tile([128, 512], F32, tag="psA")
                            nc.tensor.matmul(
                                po[:],
                                aoT[:, tt * 128:(tt + 1) * 128],
                                wot_sb[:, ot * 512:(ot + 1) * 512],
                                start=True, stop=True,
                            )
                            ob = outp.tile([128, 512], F32, tag="ob")
                            nc.vector.tensor_copy(ob[:], po[:])
                            r0 = b * S + tt * 128
                            nc.sync.dma_start(
                                out.ap()[r0:r0 + 128, ot * 512:(ot + 1) * 512],
                                ob[:])
                if dbg and b == 0:
                    nc.sync.dma_start(dbg_ao.ap(), aoT[:])
    nc.compile()
    return nc


def _causal_mask():
    # mask[r, j] = 1 where the key row r is visible to query column j
    r = np.arange(128)[:, None]
    j = np.arange(128)[None, :]
    return (r <= j).astype(np.float32)


def _maybe_register_ntff_hook():
    try:
        import antenv
        if getattr(antenv, "axon_hooks", None) is not None:
            return True
        import sys
        import types
        from trn_agent_boot.trn_boot import _ntff_profile_via_ctypes
        mod = types.ModuleType("antenv.axon_hooks")
        state = {"hook": _ntff_profile_via_ctypes("/opt/axon/libaxon_pjrt.so")}
        mod.set_axon_ntff_profile_hook = lambda h: state.__setitem__("hook", h)
        mod.get_axon_ntff_profile_hook = lambda: state["hook"]
        sys.modules["antenv.axon_hooks"] = mod
        antenv.axon_hooks = mod
        return True
    except Exception:
        return False


_NC_CACHE = {}


def kernel(x, W_qkv, W_o):
    assert x.shape == (B, S, D)
    xt = np.ascontiguousarray(
        x.reshape(B * S, D).T.astype(np.float32)).astype(np.float16)
    mask = _causal_mask().astype(np.float16)
    onesd = np.ones((128, 64), dtype=np.float32)
    in_maps = []
    for c in range(N_CORES):
        sl = slice(c * CD, (c + 1) * CD)
        wqkv_c = np.concatenate(
            [W_qkv[0 * D:][sl], W_qkv[1 * D:][sl], W_qkv[2 * D:][sl]], axis=0)
        wqkvt = np.ascontiguousarray(
            wqkv_c.T.astype(np.float32)).astype(np.float16)
        wot = np.ascontiguousarray(
            W_o[:, sl].T.astype(np.float32)).astype(np.float16)
        in_maps.append({"xt": xt, "wqkvt": wqkvt, "wot": wot, "mask": mask,
                        "onesd": onesd})

    if "nc" not in _NC_CACHE:
        _NC_CACHE["nc"] = _build_bass()
    nc = _NC_CACHE["nc"]

    trace = bool(os.environ.get("BASS_KERNEL_TRACE")) and _maybe_register_ntff_hook()
    res = run_bass_kernel_spmd(nc, in_maps, core_ids=list(range(N_CORES)),
                               trace=trace)
    if trace and res.exec_time_ns is not None:
        print(f"HW exec time: {res.exec_time_ns} ns")

    acc = np.zeros((SEQ, D), dtype=np.float64)
    for c in range(N_CORES):
        acc += res.results[c]["out"]
    return acc.astype(np.float32).reshape(B, S, D)


# revision 18
# speedup vs baseline: 1.5014x; 1.1478x over previous
"""Trainium2 Bass kernel for nn_Attention_81870666597078.

Multi-head causal self-attention (b=4, s=2048, d=1024, 16 heads) with QKV/O
projections, tensor-parallel over heads: each of the 8 NeuronCores computes
2 heads (128 of the 1024 hidden dims) end-to-end and produces a partial O
projection; the host sums the 8 partials (the "all-reduce").

Per-core dataflow (matmuls in fp16 with fp32 PSUM accumulation; the
softmax-denominator reciprocal path stays fp32):
  - QKV projection into transposed layout: qT/kT/vT [128 dims, seq] from
    xT tiles (moving) and W^T tiles (stationary).
  - vT is re-transposed on the PE into v-natural [seq, dh] tiles, stored with
    a fused ones-column ([v | 1] per k-tile) so the PV matmul also produces
    the softmax denominator as output row 64.
  - Scores are computed transposed, S^T [k, q], so the PV contraction (over
    k) needs no transposes. Score tiles come in pairs sharing a 2-bank
    [128, 1024] PSUM tile so one ACT exp covers both (amortizes the ~352-cycle
    ACTIVATE overhead). No max subtraction: scores are ~N(0,1) after the 1/8
    scale, exp cannot overflow. Causal masking is a multiplicative 0/1
    [128, 128] mask on the diagonal band of each diagonal tile; fully-masked
    column ranges are skipped in the PV matmul via slicing.
  - Normalization: reciprocal_approx_fast of the denominator row, broadcast
    across 64 partitions via a K=1 fp32 ones matmul, multiplied into the PV
    output.
  - O projection from attn^T tiles (stationary) and W_o^T tiles (moving);
    partial [seq, 1024] fp32 output DMAd out.
"""
import os

import numpy as np

import concourse.bass as bass  # noqa: F401
import concourse.mybir as mybir
from concourse import bacc
from concourse.bass_utils import run_bass_kernel_spmd
from concourse.masks import make_identity
from concourse.tile import TileContext

dt = mybir.dt
F32 = dt.float32
F16 = dt.float16
Exp = mybir.ActivationFunctionType.Exp

N_CORES = 8
B = 4
S = 2048
D = 1024
DH = 64
CD = 128          # head dims per core (2 heads x 64)
NDT = D // 128    # 8 k-tiles over the model dim
NST = S // 512    # 4 seq tiles of 512 per batch
SEQ = B * S       # 8192


def _build_bass():
    nc = bacc.Bacc("TRN2", target_bir_lowering=False, debug=False)
    xt = nc.dram_tensor("xt", [D, SEQ], F16, kind="ExternalInput")
    wqkvt = nc.dram_tensor("wqkvt", [D, 3 * CD], F16, kind="ExternalInput")
    wot = nc.dram_tensor("wot", [CD, D], F16, kind="ExternalInput")
    mask = nc.dram_tensor("mask", [128, 128], F16, kind="ExternalInput")
    onesd = nc.dram_tensor("onesd", [128, 64], F32, kind="ExternalInput")
    out = nc.dram_tensor("out", [SEQ, D], F32, kind="ExternalOutput")
    dbg = os.environ.get("BASS_KERNEL_DEBUG")
    if dbg:
        dbg_q = nc.dram_tensor("dbg_q", [128, S], F16, kind="ExternalOutput")
        dbg_k = nc.dram_tensor("dbg_k", [128, S], F16, kind="ExternalOutput")
        dbg_v = nc.dram_tensor("dbg_v", [128, 2080], F16, kind="ExternalOutput")
        dbg_ao = nc.dram_tensor("dbg_ao", [128, S], F16, kind="ExternalOutput")
        dbg_den = nc.dram_tensor("dbg_den", [8, 512], F32, kind="ExternalOutput")
        dbg_rc = nc.dram_tensor("dbg_rc", [8, 512], F32, kind="ExternalOutput")
        dbg_pr = nc.dram_tensor("dbg_pr", [128, 1024], F16, kind="ExternalOutput")

    xt_view = xt.ap().rearrange("(a p) s -> p a s", p=128)      # [128, 8, 8192]
    wq_view = wqkvt.ap().rearrange("(a p) m -> p a m", p=128)   # [128, 8, 384]

    with TileContext(nc) as tc:
        with (
            tc.tile_pool(name="const", bufs=1) as const,
            tc.tile_pool(name="perb", bufs=2) as perb,
            tc.tile_pool(name="xp", bufs=3) as xp,
            tc.tile_pool(name="probs", bufs=6) as probsp,
            tc.tile_pool(name="outp", bufs=4) as outp,
            tc.tile_pool(name="small", bufs=2) as small,
            tc.tile_pool(name="psA", bufs=2, space="PSUM") as psA,
            tc.tile_pool(name="psS", bufs=2, space="PSUM") as psS,
            tc.tile_pool(name="psPV", bufs=2, space="PSUM") as psPV,
        ):
            wq_sb = const.tile([128, NDT, 3 * CD], F16, tag="wq")
            wot_sb = const.tile([128, D], F16, tag="wot")
            mask_sb = const.tile([128, 128], F16, tag="mask")
            ident_sb = const.tile([128, 128], F32, tag="ident")
            ones_sb = const.tile([128, 64], F32, tag="ones")
            onesr_sb = const.tile([1, 64], dt.float32r, tag="onesr")
            nc.sync.dma_start(wq_sb[:], wq_view)
            nc.sync.dma_start(wot_sb[:], wot.ap())
            nc.sync.dma_start(mask_sb[:], mask.ap())
            nc.sync.dma_start(ones_sb[:], onesd.ap())
            nc.sync.dma_start(onesr_sb[:], onesd.ap()[0:1, :].bitcast(dt.float32r))
            make_identity(nc, ident_sb[:])

            for b in range(B):
                qT = perb.tile([128, S], F16, tag="qT")
                kT = perb.tile([128, S], F16, tag="kT")
                vT = perb.tile([128, S], F32, tag="vT")
                v65 = perb.tile([128, (S // 128) * 2 * 65], F16, tag="v65")
                aoT = perb.tile([128, S], F16, tag="aoT")
                # ones column of every [v | 1] group (32 groups of 65 cols)
                v65g = v65[:].rearrange("p (g c) -> p g c", c=65)
                nc.vector.tensor_copy(
                    v65g[:, :, 64:65],
                    ones_sb[:, 0:1][:, None, :].broadcast_to([128, 32, 1]))

                # ---- QKV projection (into transposed [dims, seq] layout) ----
                for st in range(NST):
                    xtile = xp.tile([128, NDT, 512], F16, tag="xt")
                    c = b * S + st * 512
                    nc.sync.dma_start(xtile[:], xt_view[:, :, c:c + 512])
                    for g, dest in ((0, qT), (1, kT), (2, vT)):
                        psp = psA.tile([128, 512], F32, tag="psA")
                        for kt in range(NDT):
                            nc.tensor.matmul(
                                psp[:],
                                wq_sb[:, kt, g * 128:(g + 1) * 128],
                                xtile[:, kt, :],
                                start=(kt == 0), stop=(kt == NDT - 1),
                            )
                        nc.vector.tensor_copy(dest[:, st * 512:(st + 1) * 512],
                                              psp[:])

                # ---- v natural layout [seq, dh] with fused ones column ----
                for t in range(S // 128):
                    pst = psA.tile([128, 128], F32, tag="psA")
                    nc.tensor.transpose(pst[:], vT[:, t * 128:(t + 1) * 128],
                                        ident_sb[:])
                    for h in (0, 1):
                        g0 = (t * 2 + h) * 65
                        nc.vector.tensor_copy(v65[:, g0:g0 + 64],
                                              pst[:, h * 64:(h + 1) * 64])

                if dbg and b == 0:
                    nc.sync.dma_start(dbg_q.ap(), qT[:])
                    nc.sync.dma_start(dbg_k.ap(), kT[:])
                    nc.sync.dma_start(dbg_v.ap(), v65[:])

                # ---- attention + O projection per 512-wide q tile ----
                for qt in range(NST):
                    for h in (0, 1):
                        pv = psPV.tile([65, 512], F32, tag="pv")
                        nkt = 4 * (qt + 1)
                        for kp in range(nkt // 2):
                            sp = psS.tile([128, 1024], F32, tag="s")
                            pr = probsp.tile([128, 1024], F16, tag="pr")
                            for kl in (0, 1):
                                kt = kp * 2 + kl
                                nc.tensor.matmul(
                                    sp[:, kl * 512:(kl + 1) * 512],
                                    kT[h * 64:(h + 1) * 64,
                                       kt * 128:(kt + 1) * 128],
                                    qT[h * 64:(h + 1) * 64,
                                       qt * 512:(qt + 1) * 512],
                                    start=True, stop=True,
                                )
                            # one exp over both k-tiles (2 PSUM banks wide)
                            nc.scalar.activation(pr[:], sp[:], Exp, scale=0.125)
                            for kl in (0, 1):
                                kt = kp * 2 + kl
                                o = kt * 128 - qt * 512
                                c0 = max(0, o)
                                if o >= 0:
                                    nc.vector.tensor_mul(
                                        pr[:, kl * 512 + o:kl * 512 + o + 128],
                                        pr[:, kl * 512 + o:kl * 512 + o + 128],
                                        mask_sb[:])
                                g0 = (kt * 2 + h) * 65
                                nc.tensor.matmul(
                                    pv[:, c0:512],
                                    v65[:, g0:g0 + 65],
                                    pr[:, kl * 512 + c0:(kl + 1) * 512],
                                    start=(kt == 0), stop=(kt == nkt - 1),
                                    skip_group_check=True,
                                )
                            if dbg and b == 0 and h == 0 and qt == 0 and kp == 0:
                                nc.sync.dma_start(dbg_pr.ap(), pr[:])
                        # normalize by the denominator (PV row 64)
                        den = small.tile([1, 512], F32, tag="den")
                        nc.vector.tensor_copy(den[:], pv[64:65, :])
                        rcf = small.tile([1, 512], F32, tag="rcf")
                        nc.vector.reciprocal_approx_fast(rcf[:], den[:])
                        rcr = small.tile([1, 512], dt.float32r, tag="rcr")
                        with nc.allow_low_precision(
                                reason="f32r recip broadcast: ~1e-4 rounding"):
                            nc.vector.tensor_copy(rcr[:], rcf[:])
                        pbc = psA.tile([64, 512], F32, tag="psA")
                        nc.tensor.matmul(pbc[:], onesr_sb[:], rcr[:],
                                         start=True, stop=True)
                        rb = small.tile([64, 512], F32, tag="rb")
                        nc.scalar.copy(rb[:], pbc[:])
                        a0, a1 = h * 64, (h + 1) * 64
                        q0, q1 = qt * 512, (qt + 1) * 512
                        nc.vector.tensor_mul(aoT[a0:a1, q0:q1], pv[0:64, :],
                                             rb[:])
                        if dbg and b == 0:
                            di = h * 4 + qt
                            nc.sync.dma_start(dbg_den.ap()[di:di + 1, :], den[:])
                            nc.sync.dma_start(dbg_rc.ap()[di:di + 1, :], rcf[:])

                    # O projection for this q block
                    for t in range(4):
                        tt = qt * 4 + t
                        for ot in range(2):
                            po = psA.tile([128, 512], F32, tag="psA")
                            nc.tensor.matmul(
                                po[:],
                                aoT[:, tt * 128:(tt + 1) * 128],
                                wot_sb[:, ot * 512:(ot + 1) * 512],
                                start=True, stop=True,
                            )
                            ob = outp.tile([128, 512], F32, tag="ob")
                            nc.vector.tensor_copy(ob[:], po[:])
                            r0 = b * S + tt * 128
                            nc.sync.dma_start(
                                out.ap()[r0:r0 + 128, ot * 512:(ot + 1) * 512],
                                ob[:])
                if dbg and b == 0:
                    nc.sync.dma_start(dbg_ao.ap(), aoT[:])
    nc.compile()
    return nc


def _causal_mask():
    # mask[r, j] = 1 where the key row r is visible to query column j
    r = np.arange(128)[:, None]
    j = np.arange(128)[None, :]
    return (r <= j).astype(np.float32)


def _maybe_register_ntff_hook():
    try:
        import antenv
        if getattr(antenv, "axon_hooks", None) is not None:
            return True
        import sys
        import types
        from trn_agent_boot.trn_boot import _ntff_profile_via_ctypes
        mod = types.ModuleType("antenv.axon_hooks")
        state = {"hook": _ntff_profile_via_ctypes("/opt/axon/libaxon_pjrt.so")}
        mod.set_axon_ntff_profile_hook = lambda h: state.__setitem__("hook", h)
        mod.get_axon_ntff_profile_hook = lambda: state["hook"]
        sys.modules["antenv.axon_hooks"] = mod
        antenv.axon_hooks = mod
        return True
    except Exception:
        return False


_NC_CACHE = {}


def kernel(x, W_qkv, W_o):
    assert x.shape == (B, S, D)
    xt = np.ascontiguousarray(
        x.reshape(B * S, D).T.astype(np.float32)).astype(np.float16)
    mask = _causal_mask().astype(np.float16)
    onesd = np.ones((128, 64), dtype=np.float32)
    in_maps = []
    for c in range(N_CORES):
        sl = slice(c * CD, (c + 1) * CD)
        wqkv_c = np.concatenate(
            [W_qkv[0 * D:][sl], W_qkv[1 * D:][sl], W_qkv[2 * D:][sl]], axis=0)
        wqkvt = np.ascontiguousarray(
            wqkv_c.T.astype(np.float32)).astype(np.float16)
        wot = np.ascontiguousarray(
            W_o[:, sl].T.astype(np.float32)).astype(np.float16)
        in_maps.append({"xt": xt, "wqkvt": wqkvt, "wot": wot, "mask": mask,
                        "onesd": onesd})

    if "nc" not in _NC_CACHE:
        _NC_CACHE["nc"] = _build_bass()
    nc = _NC_CACHE["nc"]

    trace = bool(os.environ.get("BASS_KERNEL_TRACE")) and _maybe_register_ntff_hook()
    res = run_bass_kernel_spmd(nc, in_maps, core_ids=list(range(N_CORES)),
                               trace=trace)
    if trace and res.exec_time_ns is not None:
        print(f"HW exec time: {res.exec_time_ns} ns")

    acc = np.zeros((SEQ, D), dtype=np.float64)
    for c in range(N_CORES):
        acc += res.results[c]["out"]
    return acc.astype(np.float32).reshape(B, S, D)
